# revision 19
# baseline (speedup 1.0000x reference)
"""Multi-head attention (B=4, S=2048, E=1024, H=16, D=64) on 8 TRN2 NeuronCores.

Sharding: tensor-parallel over heads -- core c computes heads 2c and 2c+1.
Each core receives the full x (cast bf16) plus its [E, 128] slices of
Wq/Wk/Wv and biases, and produces out[:, :, 128c:128c+128]; the host
concatenates along the feature dim.

Per-core dataflow (no TensorEngine transposes anywhere; HW-measured PE
model: a matmul costs ~N_out_cols x 0.42ns + ~15ns, any K<=128 -- so
always stream the SHORT output dim):
  x  --DMA-transpose-->  xT [E-chunk=128, S] (bf16)
  qT = (Wq^T xT)/8 + bq/8   [128(d,2 heads), S]   (PE + DVE psum->sbuf)
  kT =  Wk^T xT + bk        [128, S]
  vT =  Wv^T xT             [128, S] --DMA-transpose--> v_sb [keys, feats+ones]
  scoresT[sk, sq] = kT^T qT (K=64 per head, both heads packed in one
        [128,1024] PSUM tile; N=512 is the short side here)
  exp: ScalarE Exp -> ex bf16 for 12/16 key tiles; DVE int16-Schraudolph
        (bits = round(x*SCHR_A+SCHR_B) == bf16(exp x)) for tiles SCHR_I,
        keeping ACT (~1.06us/tile) off the critical path now that it
        would otherwise co-pace with the PE.
  FLIPPED PV: y[q, f] += ex^T v_aug, one matmul per (key tile, head,
        128-query subtile): lhsT = ex slice [128k, 128q] (stationary),
        rhs = v_aug [128k, 66] (moving, N=66; 42ns/mm measured).  Output
        lands QUERY-MAJOR with the denominator in column 64 -- no output
        transposes at all.  8 accumulators [128,66] live in 2 PSUM banks.
  normalize: DVE reciprocal of col 64 + per-partition scale -> ob bf16
        [128q, 4, 128(2 heads)]; one DMA out per query block.

The emission order software-pipelines batches: batch b+1's projection
matmuls are interleaved into batch b's attention loop so the
TensorEngine and ScalarE never idle.
"""

import os
import sys
import types

import numpy as np
import ml_dtypes

import concourse.bass as bass
import concourse.tile as tile
from concourse import bacc, mybir
from concourse.bass_utils import run_bass_kernel_spmd

B, S, E, H, D = 4, 2048, 1024, 16, 64
NCORES = 8
DHC = (H // NCORES) * D  # 128 feature cols per core (2 heads)
NEG = -1.0e9  # additive mask bias for masked-out keys
BF16 = mybir.dt.bfloat16
F32 = mybir.dt.float32
I16 = mybir.dt.int16
SK = S // 128  # 16 key tiles per batch
SQ = S // 512  # 4 query blocks per batch

# bf16 Schraudolph exp: bits = round(x*SCHR_A + SCHR_B) as int16 == bf16(exp x)
SCHR_A = 128.0 / float(np.log(2.0))  # 184.665...
SCHR_B = 127.0 * 128.0 - 7.42       # RNE convert measured on HW
SCHR_I = (2, 6, 10, 14)  # key tiles whose exp runs on VectorE

LAST_RESULTS = None  # BassKernelResults of the most recent kernel() call


def _install_trace_hook():
    """Register the axon NTFF-profile hook so BASS_TRACE=1 works."""
    try:
        import antenv

        if "antenv.axon_hooks" in sys.modules:
            return
        mod = types.ModuleType("antenv.axon_hooks")
        _hook = [None]
        mod.set_axon_ntff_profile_hook = lambda h: _hook.__setitem__(0, h)
        mod.get_axon_ntff_profile_hook = lambda: _hook[0]
        sys.modules["antenv.axon_hooks"] = mod
        antenv.axon_hooks = mod
        from trn_agent_boot.trn_boot import _ntff_profile_via_ctypes

        so = "/opt/axon/libaxon_pjrt.so"
        if os.path.exists(so):
            mod.set_axon_ntff_profile_hook(_ntff_profile_via_ctypes(so))
    except Exception:
        pass


_install_trace_hook()


class _Ctx:
    """Shared emission state for one core's program."""


def _setup(nc, tc, ctx, aps, has_bv, use_schr):
    s = _Ctx()
    (s.x, wq, bq, bv, s.out) = aps
    s.has_bv = has_bv
    s.use_schr = use_schr

    s.singles = ctx.enter_context(tc.tile_pool(name="singles", bufs=1))
    s.xt_pool = ctx.enter_context(tc.tile_pool(name="xt", bufs=16))
    s.qk_pool = ctx.enter_context(tc.tile_pool(name="qk", bufs=4))
    s.vt_pool = ctx.enter_context(tc.tile_pool(name="vt", bufs=2))
    s.vsb_pool = ctx.enter_context(tc.tile_pool(name="vsb", bufs=3))
    s.exp_pool = ctx.enter_context(tc.tile_pool(name="exp", bufs=8))
    s.schr_pool = ctx.enter_context(tc.tile_pool(name="schr", bufs=4))
    s.rc_pool = ctx.enter_context(tc.tile_pool(name="rc", bufs=4))
    s.ob_pool = ctx.enter_context(tc.tile_pool(name="ob", bufs=2))
    # PSUM (8 banks): scores 2x[128,1024]=4, flipped-PV accum 2x[128,4,128]
    # (one bank per head, 4 query-subtile slots each) = 2, projection
    # accum 2x[128,512]=2.
    s.ps_pool = ctx.enter_context(tc.tile_pool(name="ps", bufs=2, space="PSUM"))
    s.py_pool = ctx.enter_context(tc.tile_pool(name="py", bufs=2, space="PSUM"))
    s.prj_pool = ctx.enter_context(tc.tile_pool(name="prj", bufs=2, space="PSUM"))

    s.wcat_sb = s.singles.tile([128, 3, 8, 128], BF16, tag="wcat")
    s.consts_sb = s.singles.tile([128, 130], F32, tag="consts")
    s.warm = s.singles.tile([1, 4], F32, tag="warm")
    s.w_sb = {"wq": s.wcat_sb[:, 0], "wk": s.wcat_sb[:, 1],
              "wv": s.wcat_sb[:, 2]}
    s.wq_ap, s.bq_ap, s.bv_ap = wq, bq, bv
    if has_bv:
        s.bv_sb = s.singles.tile([128, DHC], F32, tag="bv")
    return s


def _emit_setup_dmas(nc, s):
    """Weights/consts loads + ACT exp-table warmup (emitted after the
    batch-0 xT transposes so those hit the wire first)."""
    nc.vector.memset(s.warm[:, :], 0.0)
    nc.scalar.activation(out=s.warm[0:1, 2:3], in_=s.warm[0:1, 0:1],
                         func=mybir.ActivationFunctionType.Exp)
    nc.scalar.dma_start(out=s.wcat_sb[:, :, :, :], in_=s.wq_ap)
    nc.scalar.dma_start(out=s.consts_sb[:, :], in_=s.bq_ap)
    if s.has_bv:
        bv = s.bv_ap
        bv_bcast = bass.AP(tensor=bv.tensor, offset=bv.offset,
                           ap=[[0, 128]] + bv.ap[1:])
        nc.gpsimd.dma_start(out=s.bv_sb[:, :], in_=bv_bcast)


def _gen_proj(nc, s, b, split=False):
    """Generator: emits batch b's xT loads + q/k/v projections.

    When split=True (batch 0 bootstrap) the order is
      q(j0), k/v(jh0), k/v(jh1) -> "SPLIT" -> k/v(jh2..3), q(j1..3)
    so attention(0, j=0) can start after only half the x load.
    """
    mult, add = mybir.AluOpType.mult, mybir.AluOpType.add

    qT = s.qk_pool.tile([128, S], BF16, tag="qk", name=f"qT{b}")
    kT = s.qk_pool.tile([128, S], BF16, tag="qk", name=f"kT{b}")
    vt = s.vt_pool.tile([128, S], BF16, tag="vt", name=f"vt{b}")
    v_sb = s.vsb_pool.tile([128, SK, 192], BF16, tag="vsb", name=f"v{b}")
    s.proj = getattr(s, "proj", {})
    s.proj[b] = (qT, kT, v_sb)

    xt = []
    for c in range(8):
        t = s.xt_pool.tile([128, S], BF16, tag="xt", name=f"xt{b}_{c}")
        xt.append(t)
    if split:
        # Quarter-granular, earliest-needed-first: q(j0)/k(jh0)/v(jh0) can
        # start after just the first 512-row strip.  Transpose DISPATCH is
        # ~1us serial per call on the issuing queue, so the first two
        # quarters alternate between the SP and ACT queues (the two HWDGE
        # engines) to halve time-to-issue; later quarters stay on SP so
        # the ACT queue is free for the exp chain by then.
        _emit_setup_dmas(nc, s)
        for q in range(4):
            for c in range(8):
                nc.sync.dma_start_transpose(
                    out=xt[c][:, 512 * q:512 * (q + 1)],
                    in_=s.x[b, 512 * q:512 * (q + 1), 128 * c:128 * (c + 1)])
    else:
        for c in range(8):
            nc.sync.dma_start_transpose(
                out=xt[c][:, :], in_=s.x[b, :, 128 * c:128 * (c + 1)])
    # v_sb cols: [v0(0:64) | ones(64:66) | pad | v1(80:144) | ones(144:146)]
    nc.vector.memset(v_sb[:, :, 64:66], 1.0)
    nc.vector.memset(v_sb[:, :, 144:146], 1.0)
    yield "c"

    def q_or_k(name, dest, bias_col, scale, jhs):
        w = s.w_sb[name]
        for jh in jhs:
            ps = s.prj_pool.tile([128, 512], F32, tag="prj", name="pj")
            for c in range(8):
                nc.tensor.matmul(
                    ps[:, :], w[:, c, :], xt[c][:, 512 * jh:512 * (jh + 1)],
                    start=(c == 0), stop=(c == 7))
                if c % 2 == 1:
                    yield "c"
            nc.vector.tensor_scalar(
                out=dest[:, 512 * jh:512 * (jh + 1)], in0=ps[:, :],
                scalar1=scale,
                scalar2=s.consts_sb[:, bias_col:bias_col + 1],
                op0=mult, op1=add)
            yield "c"

    def v_proj(jh):
        w = s.w_sb["wv"]
        ps = s.prj_pool.tile([128, 512], F32, tag="prj", name="pv")
        for c in range(8):
            nc.tensor.matmul(
                ps[:, :], w[:, c, :], xt[c][:, 512 * jh:512 * (jh + 1)],
                start=(c == 0), stop=(c == 7))
            if c % 2 == 1:
                yield "c"
        nc.vector.tensor_copy(out=vt[:, 512 * jh:512 * (jh + 1)], in_=ps[:, :])
        yield "c"

    def v_transpose(jh_lo, jh_hi):
        # vT [feats, q-range] -> v_sb[key, tile, feat]; out offsets must be
        # multiples of 16 elements, in partition base 0 or 64 only.
        qsl = slice(512 * jh_lo, 512 * (jh_hi + 1))
        tsl = slice(4 * jh_lo, 4 * (jh_hi + 1))
        nc.sync.dma_start_transpose(out=v_sb[:, tsl, 0:64], in_=vt[0:64, qsl])
        nc.sync.dma_start_transpose(out=v_sb[:, tsl, 80:144],
                                    in_=vt[64:128, qsl])

    if split:
        yield from q_or_k("wq", qT, 0, 0.125, [0])
        yield from q_or_k("wk", kT, 1, 1.0, [0])
        yield from v_proj(0)
        v_transpose(0, 0)
        yield "SPLIT"
        for jh in range(1, 4):
            yield from q_or_k("wk", kT, 1, 1.0, [jh])
            yield from v_proj(jh)
            v_transpose(jh, jh)
        yield from q_or_k("wq", qT, 0, 0.125, [1, 2, 3])
    else:
        yield from q_or_k("wq", qT, 0, 0.125, [0])
        for jh in range(4):
            yield from q_or_k("wk", kT, 1, 1.0, [jh])
            yield from v_proj(jh)
            if jh % 2 == 1:
                v_transpose(jh - 1, jh)
        yield from q_or_k("wq", qT, 0, 0.125, [1, 2, 3])


def _emit_norm(nc, s, b, j, py):
    """Normalize + store query block j: DVE recip of the denominator
    column, per-partition scale to bf16, one DMA out.  Output is already
    query-major -- no transposes."""
    mult = mybir.AluOpType.mult
    rc = s.rc_pool.tile([128, 2, 4, 1], F32, tag="rc", name="rc")
    ob = s.ob_pool.tile([128, 4, DHC], BF16, tag="ob", name="ob")
    for h in range(2):
        nc.vector.reciprocal(rc[:, h, :, :], py[h][:, :, 64:65])
    for h in range(2):
        for qs in range(4):
            nc.vector.tensor_scalar(
                out=ob[:, qs, 64 * h:64 * (h + 1)], in0=py[h][:, qs, 0:64],
                scalar1=rc[:, h, qs, :], scalar2=None, op0=mult)
    if s.has_bv:
        for qs in range(4):
            nc.vector.tensor_add(ob[:, qs, :], ob[:, qs, :], s.bv_sb[:, :])
    nc.gpsimd.dma_start(
        out=s.out[b, 512 * j:512 * (j + 1), :].rearrange(
            "(c p) d -> p c d", p=128),
        in_=ob[:, 0:4, :])


def _mk_pv(nc, s, b, j, i, holder, v_sb, ex):
    """Closure emitting tile i's FLIPPED PV matmuls: lhsT = ex [128k,128q]
    slice (stationary), rhs = v_aug [128k, 66] (moving) -> py[h][q, qs, f]
    accumulates query-major.  The last one (i==SK-1) emits the normalize
    inline (it must precede the next block's PV start, which re-zeroes
    the py banks)."""

    def emit():
        if i == 0:
            # 4 accumulation groups share each bank: the HW's zero-on-start
            # arming is not per-region, so interleaved start=True groups in
            # one bank corrupt each other (probed).  Zero explicitly and
            # accumulate with start=False throughout.
            holder["py"] = [
                s.py_pool.tile([128, 4, 128], F32, tag="py", name=f"py{h}")
                for h in range(2)]
            for h in range(2):
                nc.vector.memset(holder["py"][h][:, :, 0:66], 0.0)
        py = holder["py"]
        for h in range(2):
            fsl = slice(80 * h, 80 * h + 66)
            for qs in range(4):
                nc.tensor.matmul(
                    py[h][:, qs, 0:66],
                    ex[:, 512 * h + 128 * qs:512 * h + 128 * (qs + 1)],
                    v_sb[:, i, fsl],
                    start=False, stop=(i == SK - 1))
        if i == SK - 1:
            _emit_norm(nc, s, b, j, py)
    emit.is_kt0 = (i == 0)
    return emit


def _gen_attn(nc, s, b):
    """Generator: emits batch b's attention; yields after each sk tile.

    PV matmuls run through a 3-deep carry queue so the trailing PVs of a
    (b, j) block interleave into the next block's score/exp slots instead
    of bunching up on the PE queue at the boundary."""
    from collections import deque

    qT, kT, v_sb = s.proj[b]
    if not hasattr(s, "pvq"):
        s.pvq = deque()

    for j in range(SQ):
        jsl = slice(512 * j, 512 * (j + 1))
        holder = {}
        for i in range(SK):
            pw = s.ps_pool.tile([128, 1024], F32, tag="ps", name="psc")
            for h in range(2):
                hp = slice(64 * h, 64 * (h + 1))
                nc.tensor.matmul(
                    pw[:, 512 * h:512 * (h + 1)],
                    kT[hp, 128 * i:128 * (i + 1)], qT[hp, jsl],
                    start=True, stop=True)
            if s.use_schr and i in SCHR_I:
                # DVE int16 Schraudolph keeps these tiles off the ACT chain
                ei = s.schr_pool.tile([128, 1024], I16, tag="schr", name="exs")
                nc.vector.tensor_scalar(
                    out=ei[:, :], in0=pw[:, :], scalar1=SCHR_A,
                    scalar2=s.consts_sb[:, 66 + 16 * b + i:67 + 16 * b + i],
                    op0=mybir.AluOpType.mult, op1=mybir.AluOpType.add)
                ex = ei[:, :].bitcast(BF16)
            else:
                ei = s.exp_pool.tile([128, 1024], BF16, tag="exp", name="ex")
                nc.scalar.activation(
                    out=ei[:, :], in_=pw[:, :],
                    func=mybir.ActivationFunctionType.Exp,
                    bias=s.consts_sb[:, 2 + 16 * b + i:3 + 16 * b + i],
                    scale=1.0)
                ex = ei[:, :]
            s.pvq.append(_mk_pv(nc, s, b, j, i, holder, v_sb, ex))
            # A block's first PV (start of new accumulation) waits on the
            # previous block's normalize reads + bank memsets on DVE; give
            # that chain two extra slots of queue delay so the PE never
            # stalls on it at the j boundary.
            lim = 5 if (s.pvq and getattr(s.pvq[0], "is_kt0", False)) else 3
            if len(s.pvq) > lim:
                s.pvq.popleft()()
            yield
    if b == B - 1:
        while s.pvq:
            s.pvq.popleft()()


def _emit_body(nc, tc, ctx, aps, has_bv, use_schr):
    from collections import deque

    s = _setup(nc, tc, ctx, aps, has_bv, use_schr)
    pending = deque()
    gp0 = _gen_proj(nc, s, 0, split=True)
    for tok in gp0:
        if tok == "SPLIT":
            break
    pending.append(gp0)
    first = [gp0]  # drain batch-0 leftovers at 4x so attention(0, i) never
    # outruns the k/v blocks it reads (emission order defines deps)
    for b in range(B):
        if b + 1 < B:
            pending.append(_gen_proj(nc, s, b + 1))
        for _ in _gen_attn(nc, s, b):
            n = 3 if (pending and first and pending[0] is first[0]) else 1
            for _ in range(n):
                if pending and next(pending[0], None) is None:
                    pending.popleft()
                    first.clear()
        while pending:  # proj(b+1) must be fully emitted before ga(b+1)
            if next(pending[0], None) is None:
                pending.popleft()


def _build(has_bv, use_schr):
    from contextlib import ExitStack

    nc = bacc.Bacc("TRN2", target_bir_lowering=False, debug=False)
    x = nc.dram_tensor("x", [B, S, E], BF16, kind="ExternalInput").ap()
    wq = nc.dram_tensor("wcat", [128, 3, 8, 128], BF16,
                        kind="ExternalInput").ap()
    bq = nc.dram_tensor("consts", [128, 130], F32, kind="ExternalInput").ap()
    bv = nc.dram_tensor("bv", [1, DHC], F32, kind="ExternalInput").ap()
    out = nc.dram_tensor("out", [B, S, DHC], BF16, kind="ExternalOutput").ap()
    aps = (x, wq, bq, bv, out)
    with tile.TileContext(nc) as tc:
        with ExitStack() as ctx:
            _emit_body(nc, tc, ctx, aps, has_bv, use_schr)
    nc.compile()
    return nc


_BUILD_CACHE = {}


def _get_built(has_bv, use_schr):
    key = (has_bv, use_schr)
    if key not in _BUILD_CACHE:
        _BUILD_CACHE[key] = _build(has_bv, use_schr)
    return _BUILD_CACHE[key]


def kernel(x, mask, Wq, bq, Wk, bk, Wv, bv):
    global LAST_RESULTS
    bf16 = ml_dtypes.bfloat16
    x_bf = np.asarray(x, dtype=np.float32).astype(bf16)
    mask_f = np.asarray(mask).astype(np.float32)
    maskb = (mask_f - 1.0) * (-NEG)  # 0 where mask==1, NEG where mask==0
    maskb = np.ascontiguousarray(
        maskb.reshape(B, S // 128, 128).transpose(2, 0, 1)).astype(np.float32)

    has_bv = bool(np.any(np.asarray(bv) != 0))
    # With flipped PV the PE floor drops to ~70us/batch and the pure-ACT
    # exp chain (~68us/batch) would co-pace; offloading SCHR_I tiles to a
    # DVE Schraudolph keeps ACT comfortably off the critical path at a
    # ~1e-2 rel-error cost (budget 2e-2).
    use_schr = True
    nc = _get_built(has_bv, use_schr)

    in_maps = []
    for c in range(NCORES):
        sl = slice(DHC * c, DHC * (c + 1))

        def warr(w):
            w = np.asarray(w, dtype=np.float32)[:, sl].astype(bf16)
            return np.ascontiguousarray(
                w.reshape(8, 128, 128).transpose(1, 0, 2))

        wcat = np.stack([warr(Wq), warr(Wk), warr(Wv)], axis=1)
        consts = np.empty((128, 130), dtype=np.float32)
        consts[:, 0] = np.asarray(bq, dtype=np.float32)[sl] / 8.0
        consts[:, 1] = np.asarray(bk, dtype=np.float32)[sl]
        consts[:, 2:66] = maskb.reshape(128, 64)
        consts[:, 66:130] = SCHR_B + maskb.reshape(128, 64) * SCHR_A
        in_maps.append({
            "x": x_bf,
            "wcat": np.ascontiguousarray(wcat),
            "consts": consts,
            "bv": np.ascontiguousarray(
                np.asarray(bv, dtype=np.float32)[sl].reshape(1, DHC)),
        })

    res = run_bass_kernel_spmd(nc, in_maps, core_ids=list(range(NCORES)))
    LAST_RESULTS = res
    return np.concatenate(
        [res.results[c]["out"].astype(np.float32) for c in range(NCORES)],
        axis=-1)



# revision 25
# speedup vs baseline: 1.0721x; 1.0721x over previous
"""Multi-head attention (B=4, S=2048, E=1024, H=16, D=64) on 8 TRN2 NeuronCores.

Sharding: tensor-parallel over heads -- core c computes heads 2c and 2c+1.
Each core receives the full x (cast bf16) plus its [E, 128] slices of
Wq/Wk/Wv and biases, and produces out[:, :, 128c:128c+128]; the host
concatenates along the feature dim.

Per-core dataflow (no TensorEngine transposes anywhere; HW-measured PE
model: a matmul costs ~N_out_cols x 0.42ns + ~15ns, any K<=128 -- so
always stream the SHORT output dim):
  x  --DMA-transpose-->  xT [E-chunk=128, S] (bf16)
  qT = (Wq^T xT)/8 + bq/8   [128(d,2 heads), S]   (PE + DVE psum->sbuf)
  kT =  Wk^T xT + bk        [128, S]
  vT =  Wv^T xT             [128, S] --DMA-transpose--> v_sb [keys, feats+ones]
  scoresT[sk, sq] = kT^T qT (K=64 per head, both heads packed in one
        [128,1024] PSUM tile; N=512 is the short side here)
  exp: ScalarE Exp -> ex bf16 for 12/16 key tiles; DVE int16-Schraudolph
        (bits = round(x*SCHR_A+SCHR_B) == bf16(exp x)) for tiles SCHR_I,
        keeping ACT (~1.06us/tile) off the critical path now that it
        would otherwise co-pace with the PE.
  FLIPPED PV: y[q, f] += ex^T v_aug, one matmul per (key tile, head,
        128-query subtile): lhsT = ex slice [128k, 128q] (stationary),
        rhs = v_aug [128k, 66] (moving, N=66; 42ns/mm measured).  Output
        lands QUERY-MAJOR with the denominator in column 64 -- no output
        transposes at all.  8 accumulators [128,66] live in 2 PSUM banks.
  normalize: DVE reciprocal of col 64 + per-partition scale -> ob bf16
        [128q, 4, 128(2 heads)]; one DMA out per query block.

The emission order software-pipelines batches: batch b+1's projection
matmuls are interleaved into batch b's attention loop so the
TensorEngine and ScalarE never idle.
"""

import os
import sys
import types

import numpy as np
import ml_dtypes

import concourse.bass as bass
import concourse.tile as tile
from concourse import bacc, mybir
from concourse.bass_utils import run_bass_kernel_spmd

B, S, E, H, D = 4, 2048, 1024, 16, 64
NCORES = 8
DHC = (H // NCORES) * D  # 128 feature cols per core (2 heads)
NEG = -1.0e9  # additive mask bias for masked-out keys
BF16 = mybir.dt.bfloat16
F32 = mybir.dt.float32
I16 = mybir.dt.int16
SK = S // 128  # 16 key tiles per batch
SQ = S // 512  # 4 query blocks per batch

# bf16 Schraudolph exp: bits = round(x*SCHR_A + SCHR_B) as int16 == bf16(exp x)
SCHR_A = 128.0 / float(np.log(2.0))  # 184.665...
SCHR_B = 127.0 * 128.0 - 7.42       # RNE convert measured on HW
SCHR_I = (2, 6, 10, 14)  # key tiles whose exp runs on VectorE
# Last batch has no next-batch projection work to interleave, so the PE
# outruns the ACT exp chain there; offload more tiles to DVE for it.
SCHR_I_LAST = (1, 4, 6, 9, 11, 14)

LAST_RESULTS = None  # BassKernelResults of the most recent kernel() call


def _install_trace_hook():
    """Register the axon NTFF-profile hook so BASS_TRACE=1 works."""
    try:
        import antenv

        if "antenv.axon_hooks" in sys.modules:
            return
        mod = types.ModuleType("antenv.axon_hooks")
        _hook = [None]
        mod.set_axon_ntff_profile_hook = lambda h: _hook.__setitem__(0, h)
        mod.get_axon_ntff_profile_hook = lambda: _hook[0]
        sys.modules["antenv.axon_hooks"] = mod
        antenv.axon_hooks = mod
        from trn_agent_boot.trn_boot import _ntff_profile_via_ctypes

        so = "/opt/axon/libaxon_pjrt.so"
        if os.path.exists(so):
            mod.set_axon_ntff_profile_hook(_ntff_profile_via_ctypes(so))
    except Exception:
        pass


_install_trace_hook()


class _Ctx:
    """Shared emission state for one core's program."""


def _setup(nc, tc, ctx, aps, has_bv, use_schr):
    s = _Ctx()
    (s.x, wq, bq, bv, s.out) = aps
    s.has_bv = has_bv
    s.use_schr = use_schr

    s.singles = ctx.enter_context(tc.tile_pool(name="singles", bufs=1))
    s.xt_pool = ctx.enter_context(tc.tile_pool(name="xt", bufs=16))
    s.qk_pool = ctx.enter_context(tc.tile_pool(name="qk", bufs=4))
    s.vt_pool = ctx.enter_context(tc.tile_pool(name="vt", bufs=2))
    s.vsb_pool = ctx.enter_context(tc.tile_pool(name="vsb", bufs=3))
    s.exp_pool = ctx.enter_context(tc.tile_pool(name="exp", bufs=8))
    s.schr_pool = ctx.enter_context(tc.tile_pool(name="schr", bufs=4))
    s.rc_pool = ctx.enter_context(tc.tile_pool(name="rc", bufs=4))
    s.ob_pool = ctx.enter_context(tc.tile_pool(name="ob", bufs=2))
    # PSUM (8 banks): scores 2x[128,1024]=4, flipped-PV accum 2x[128,4,128]
    # (one bank per head, 4 query-subtile slots each) = 2, projection
    # accum 2x[128,512]=2.
    s.ps_pool = ctx.enter_context(tc.tile_pool(name="ps", bufs=2, space="PSUM"))
    s.py_pool = ctx.enter_context(tc.tile_pool(name="py", bufs=2, space="PSUM"))
    s.prj_pool = ctx.enter_context(tc.tile_pool(name="prj", bufs=2, space="PSUM"))

    s.wcat_sb = s.singles.tile([128, 3, 8, 128], BF16, tag="wcat")
    s.consts_sb = s.singles.tile([128, 130], F32, tag="consts")
    s.warm = s.singles.tile([1, 4], F32, tag="warm")
    s.w_sb = {"wq": s.wcat_sb[:, 0], "wk": s.wcat_sb[:, 1],
              "wv": s.wcat_sb[:, 2]}
    s.wq_ap, s.bq_ap, s.bv_ap = wq, bq, bv
    if has_bv:
        s.bv_sb = s.singles.tile([128, DHC], F32, tag="bv")
    return s


def _emit_setup_dmas(nc, s):
    """Weights/consts loads + ACT exp-table warmup (emitted after the
    batch-0 xT transposes so those hit the wire first)."""
    nc.vector.memset(s.warm[:, :], 0.0)
    nc.scalar.activation(out=s.warm[0:1, 2:3], in_=s.warm[0:1, 0:1],
                         func=mybir.ActivationFunctionType.Exp)
    nc.scalar.dma_start(out=s.wcat_sb[:, :, :, :], in_=s.wq_ap)
    nc.scalar.dma_start(out=s.consts_sb[:, :], in_=s.bq_ap)
    if s.has_bv:
        bv = s.bv_ap
        bv_bcast = bass.AP(tensor=bv.tensor, offset=bv.offset,
                           ap=[[0, 128]] + bv.ap[1:])
        nc.gpsimd.dma_start(out=s.bv_sb[:, :], in_=bv_bcast)


def _gen_proj(nc, s, b, split=False):
    """Generator: emits batch b's xT loads + q/k/v projections.

    When split=True (batch 0 bootstrap) the order is
      q(j0), k/v(jh0), k/v(jh1) -> "SPLIT" -> k/v(jh2..3), q(j1..3)
    so attention(0, j=0) can start after only half the x load.
    """
    mult, add = mybir.AluOpType.mult, mybir.AluOpType.add

    qT = s.qk_pool.tile([128, S], BF16, tag="qk", name=f"qT{b}")
    kT = s.qk_pool.tile([128, S], BF16, tag="qk", name=f"kT{b}")
    vt = s.vt_pool.tile([128, S], BF16, tag="vt", name=f"vt{b}")
    v_sb = s.vsb_pool.tile([128, SK, 192], BF16, tag="vsb", name=f"v{b}")
    s.proj = getattr(s, "proj", {})
    s.proj[b] = (qT, kT, v_sb)

    xt = []
    for c in range(8):
        t = s.xt_pool.tile([128, S], BF16, tag="xt", name=f"xt{b}_{c}")
        xt.append(t)
    if split:
        # Quarter-granular, earliest-needed-first: q(j0)/k(jh0)/v(jh0) can
        # start after just the first 512-row strip.  Transpose DISPATCH is
        # ~1us serial per call on the issuing queue, so the first two
        # quarters alternate between the SP and ACT queues (the two HWDGE
        # engines) to halve time-to-issue; later quarters stay on SP so
        # the ACT queue is free for the exp chain by then.
        _emit_setup_dmas(nc, s)
        for q in range(2):
            for c in range(8):
                nc.sync.dma_start_transpose(
                    out=xt[c][:, 1024 * q:1024 * (q + 1)],
                    in_=s.x[b, 1024 * q:1024 * (q + 1), 128 * c:128 * (c + 1)])
    else:
        for c in range(8):
            nc.sync.dma_start_transpose(
                out=xt[c][:, :], in_=s.x[b, :, 128 * c:128 * (c + 1)])
    # v_sb cols: [v0(0:64) | ones(64:66) | pad | v1(80:144) | ones(144:146)]
    nc.vector.memset(v_sb[:, :, 64:66], 1.0)
    nc.vector.memset(v_sb[:, :, 144:146], 1.0)
    yield "c"

    def q_or_k(name, dest, bias_col, scale, jhs):
        w = s.w_sb[name]
        for jh in jhs:
            ps = s.prj_pool.tile([128, 512], F32, tag="prj", name="pj")
            for c in range(8):
                nc.tensor.matmul(
                    ps[:, :], w[:, c, :], xt[c][:, 512 * jh:512 * (jh + 1)],
                    start=(c == 0), stop=(c == 7))
                if c % 2 == 1:
                    yield "c"
            nc.vector.tensor_scalar(
                out=dest[:, 512 * jh:512 * (jh + 1)], in0=ps[:, :],
                scalar1=scale,
                scalar2=s.consts_sb[:, bias_col:bias_col + 1],
                op0=mult, op1=add)
            yield "c"

    def v_proj(jh):
        w = s.w_sb["wv"]
        ps = s.prj_pool.tile([128, 512], F32, tag="prj", name="pv")
        for c in range(8):
            nc.tensor.matmul(
                ps[:, :], w[:, c, :], xt[c][:, 512 * jh:512 * (jh + 1)],
                start=(c == 0), stop=(c == 7))
            if c % 2 == 1:
                yield "c"
        nc.vector.tensor_copy(out=vt[:, 512 * jh:512 * (jh + 1)], in_=ps[:, :])
        yield "c"

    def v_transpose(jh_lo, jh_hi):
        # vT [feats, q-range] -> v_sb[key, tile, feat]; out offsets must be
        # multiples of 16 elements, in partition base 0 or 64 only.
        qsl = slice(512 * jh_lo, 512 * (jh_hi + 1))
        tsl = slice(4 * jh_lo, 4 * (jh_hi + 1))
        nc.sync.dma_start_transpose(out=v_sb[:, tsl, 0:64], in_=vt[0:64, qsl])
        nc.sync.dma_start_transpose(out=v_sb[:, tsl, 80:144],
                                    in_=vt[64:128, qsl])

    if split:
        yield from q_or_k("wq", qT, 0, 0.125, [0])
        yield from q_or_k("wk", kT, 1, 1.0, [0])
        yield from v_proj(0)
        v_transpose(0, 0)
        yield "SPLIT"
        # jh1 + q(j1) need only the first x half -- emit them before the
        # jh2/3 blocks that wait on the second half, so the in-order PE
        # queue never blocks on ready work behind a pending DMA.
        for jh in range(1, 4):
            yield from q_or_k("wk", kT, 1, 1.0, [jh])
            yield from v_proj(jh)
            v_transpose(jh, jh)
            yield from q_or_k("wq", qT, 0, 0.125, [jh])
    else:
        yield from q_or_k("wq", qT, 0, 0.125, [0])
        for jh in range(4):
            yield from q_or_k("wk", kT, 1, 1.0, [jh])
            yield from v_proj(jh)
            if jh % 2 == 1:
                v_transpose(jh - 1, jh)
        yield from q_or_k("wq", qT, 0, 0.125, [1, 2, 3])


def _emit_norm(nc, s, b, j, py):
    """Normalize + store query block j: DVE recip of the denominator
    column, per-partition scale to bf16, one DMA out.  Output is already
    query-major -- no transposes."""
    mult = mybir.AluOpType.mult
    rc = s.rc_pool.tile([128, 2, 4, 1], F32, tag="rc", name="rc")
    ob = s.ob_pool.tile([128, 4, DHC], BF16, tag="ob", name="ob")
    for h in range(2):
        nc.vector.reciprocal(rc[:, h, :, :], py[h][:, :, 64:65])
    for h in range(2):
        for qs in range(4):
            nc.vector.tensor_scalar(
                out=ob[:, qs, 64 * h:64 * (h + 1)], in0=py[h][:, qs, 0:64],
                scalar1=rc[:, h, qs, :], scalar2=None, op0=mult)
    if s.has_bv:
        for qs in range(4):
            nc.vector.tensor_add(ob[:, qs, :], ob[:, qs, :], s.bv_sb[:, :])
    nc.gpsimd.dma_start(
        out=s.out[b, 512 * j:512 * (j + 1), :].rearrange(
            "(c p) d -> p c d", p=128),
        in_=ob[:, 0:4, :])


def _mk_pv(nc, s, b, j, i, holder, v_sb, ex):
    """Closure emitting tile i's FLIPPED PV matmuls: lhsT = ex [128k,128q]
    slice (stationary), rhs = v_aug [128k, 66] (moving) -> py[h][q, qs, f]
    accumulates query-major.  The last one (i==SK-1) emits the normalize
    inline (it must precede the next block's PV start, which re-zeroes
    the py banks)."""

    def emit():
        if i == 0:
            # 4 accumulation groups share each bank: the HW's zero-on-start
            # arming is not per-region, so interleaved start=True groups in
            # one bank corrupt each other (probed).  Zero explicitly and
            # accumulate with start=False throughout.
            holder["py"] = [
                s.py_pool.tile([128, 4, 128], F32, tag="py", name=f"py{h}")
                for h in range(2)]
            for h in range(2):
                nc.vector.memset(holder["py"][h][:, :, 0:65], 0.0)
        py = holder["py"]
        for h in range(2):
            fsl = slice(80 * h, 80 * h + 65)
            for qs in range(4):
                nc.tensor.matmul(
                    py[h][:, qs, 0:65],
                    ex[:, 512 * h + 128 * qs:512 * h + 128 * (qs + 1)],
                    v_sb[:, i, fsl],
                    start=False, stop=(i == SK - 1))
        if i == SK - 1:
            _emit_norm(nc, s, b, j, py)
    return emit


def _gen_attn(nc, s, b):
    """Generator: emits batch b's attention; yields after each sk tile.

    PV matmuls run through a 3-deep carry queue so the trailing PVs of a
    (b, j) block interleave into the next block's score/exp slots instead
    of bunching up on the PE queue at the boundary."""
    from collections import deque

    qT, kT, v_sb = s.proj[b]
    if not hasattr(s, "pvq"):
        s.pvq = deque()

    schr_set = SCHR_I_LAST if b == B - 1 else SCHR_I
    for j in range(SQ):
        jsl = slice(512 * j, 512 * (j + 1))
        holder = {}
        for i in range(SK):
            pw = s.ps_pool.tile([128, 1024], F32, tag="ps", name="psc")
            for h in range(2):
                hp = slice(64 * h, 64 * (h + 1))
                nc.tensor.matmul(
                    pw[:, 512 * h:512 * (h + 1)],
                    kT[hp, 128 * i:128 * (i + 1)], qT[hp, jsl],
                    start=True, stop=True)
            if s.use_schr and i in schr_set:
                # DVE int16 Schraudolph keeps these tiles off the ACT chain
                ei = s.schr_pool.tile([128, 1024], I16, tag="schr", name="exs")
                nc.vector.tensor_scalar(
                    out=ei[:, :], in0=pw[:, :], scalar1=SCHR_A,
                    scalar2=s.consts_sb[:, 66 + 16 * b + i:67 + 16 * b + i],
                    op0=mybir.AluOpType.mult, op1=mybir.AluOpType.add)
                ex = ei[:, :].bitcast(BF16)
            else:
                ei = s.exp_pool.tile([128, 1024], BF16, tag="exp", name="ex")
                nc.scalar.activation(
                    out=ei[:, :], in_=pw[:, :],
                    func=mybir.ActivationFunctionType.Exp,
                    bias=s.consts_sb[:, 2 + 16 * b + i:3 + 16 * b + i],
                    scale=1.0)
                ex = ei[:, :]
            s.pvq.append(_mk_pv(nc, s, b, j, i, holder, v_sb, ex))
            if len(s.pvq) > 3:
                s.pvq.popleft()()
            yield
    if b == B - 1:
        while s.pvq:
            s.pvq.popleft()()


def _emit_body(nc, tc, ctx, aps, has_bv, use_schr):
    from collections import deque

    s = _setup(nc, tc, ctx, aps, has_bv, use_schr)
    pending = deque()
    gp0 = _gen_proj(nc, s, 0, split=True)
    for tok in gp0:
        if tok == "SPLIT":
            break
    pending.append(gp0)
    first = [gp0]  # drain batch-0 leftovers at 4x so attention(0, i) never
    # outruns the k/v blocks it reads (emission order defines deps)
    for b in range(B):
        if b + 1 < B:
            pending.append(_gen_proj(nc, s, b + 1))
        for _ in _gen_attn(nc, s, b):
            n = 3 if (pending and first and pending[0] is first[0]) else 1
            for _ in range(n):
                if pending and next(pending[0], None) is None:
                    pending.popleft()
                    first.clear()
        while pending:  # proj(b+1) must be fully emitted before ga(b+1)
            if next(pending[0], None) is None:
                pending.popleft()


def _build(has_bv, use_schr):
    from contextlib import ExitStack

    nc = bacc.Bacc("TRN2", target_bir_lowering=False, debug=False)
    x = nc.dram_tensor("x", [B, S, E], BF16, kind="ExternalInput").ap()
    wq = nc.dram_tensor("wcat", [128, 3, 8, 128], BF16,
                        kind="ExternalInput").ap()
    bq = nc.dram_tensor("consts", [128, 130], F32, kind="ExternalInput").ap()
    bv = nc.dram_tensor("bv", [1, DHC], F32, kind="ExternalInput").ap()
    out = nc.dram_tensor("out", [B, S, DHC], BF16, kind="ExternalOutput").ap()
    aps = (x, wq, bq, bv, out)
    with tile.TileContext(nc) as tc:
        with ExitStack() as ctx:
            _emit_body(nc, tc, ctx, aps, has_bv, use_schr)
    nc.compile()
    return nc


_BUILD_CACHE = {}


def _get_built(has_bv, use_schr):
    key = (has_bv, use_schr)
    if key not in _BUILD_CACHE:
        _BUILD_CACHE[key] = _build(has_bv, use_schr)
    return _BUILD_CACHE[key]


def kernel(x, mask, Wq, bq, Wk, bk, Wv, bv):
    global LAST_RESULTS
    bf16 = ml_dtypes.bfloat16
    x_bf = np.asarray(x, dtype=np.float32).astype(bf16)
    mask_f = np.asarray(mask).astype(np.float32)
    maskb = (mask_f - 1.0) * (-NEG)  # 0 where mask==1, NEG where mask==0
    maskb = np.ascontiguousarray(
        maskb.reshape(B, S // 128, 128).transpose(2, 0, 1)).astype(np.float32)

    has_bv = bool(np.any(np.asarray(bv) != 0))
    # With flipped PV the PE floor drops to ~70us/batch and the pure-ACT
    # exp chain (~68us/batch) would co-pace; offloading SCHR_I tiles to a
    # DVE Schraudolph keeps ACT comfortably off the critical path at a
    # ~1e-2 rel-error cost (budget 2e-2).
    use_schr = True
    nc = _get_built(has_bv, use_schr)

    in_maps = []
    for c in range(NCORES):
        sl = slice(DHC * c, DHC * (c + 1))

        def warr(w):
            w = np.asarray(w, dtype=np.float32)[:, sl].astype(bf16)
            return np.ascontiguousarray(
                w.reshape(8, 128, 128).transpose(1, 0, 2))

        wcat = np.stack([warr(Wq), warr(Wk), warr(Wv)], axis=1)
        consts = np.empty((128, 130), dtype=np.float32)
        consts[:, 0] = np.asarray(bq, dtype=np.float32)[sl] / 8.0
        consts[:, 1] = np.asarray(bk, dtype=np.float32)[sl]
        consts[:, 2:66] = maskb.reshape(128, 64)
        consts[:, 66:130] = SCHR_B + maskb.reshape(128, 64) * SCHR_A
        in_maps.append({
            "x": x_bf,
            "wcat": np.ascontiguousarray(wcat),
            "consts": consts,
            "bv": np.ascontiguousarray(
                np.asarray(bv, dtype=np.float32)[sl].reshape(1, DHC)),
        })

    res = run_bass_kernel_spmd(nc, in_maps, core_ids=list(range(NCORES)))
    LAST_RESULTS = res
    return np.concatenate(
        [res.results[c]["out"].astype(np.float32) for c in range(NCORES)],
        axis=-1)

